# revision 17
# baseline (speedup 1.0000x reference)
"""TRN2 Bass kernel for nn_CombCrossAttention (GQA cross-attention block).

Computation (T=2048, K=2048, E=4096, H=32 q-heads, KVH=8 kv-heads, D=128):
    q  = hidden @ Wq.T;  per-head RMSNorm(q) * q_norm_w
    kn = RMSNorm(k) * k_norm_w  (GQA: each kv head serves 4 q heads)
    attn = softmax(qn @ kn.T / sqrt(D)) @ v
    out  = attn @ Wo.T

Sharding: tensor-parallel over heads on 8 NeuronCores. Core c owns q-heads
4c..4c+3 (Wq rows 512c..512c+512) and kv-head c, plus Wo columns
512c..512c+512; each core emits a [T, E] partial of the o-projection and
the host sums the 8 partials (the "all-reduce").

Device-side layout trick: everything is computed transposed ([feature, t])
so no on-chip transposes are needed anywhere:
  - q-proj emits qT [e', t] directly (lhsT = Wq shard already transposed)
  - scores are computed as scoresT [kk, t]; softmax's reductions over kk
    become ones-vector matmuls on the PE (cross-partition sums)
  - k-side RMSNorm, both norm weights and 1/sqrt(D) are folded into a
    host-precomputed k''; q-side RMSNorm becomes a per-t scale computed
    on-device from a ones-matmul + sqrt + reciprocal, applied in place
  - exp() needs no max-subtraction: post-RMSNorm scores are O(1)-bounded
Matmuls run in float32r (full PE rate, ~1.5e-4 max rel err).
"""
import sys

sys.path.insert(0, "/opt/trn_rl_repo")

import numpy as np

import jax
try:
    jax.config.update("jax_compilation_cache_dir", "/tmp/jax_neff_cache")
    jax.config.update("jax_persistent_cache_min_compile_time_secs", 1.0)
except Exception:
    pass

import concourse.bass as bass  # noqa: F401
import concourse.mybir as mybir
import concourse.tile as tile
from concourse import bacc, bass_utils

EPS = 1e-5
T, K, E, H, KVH, D = 2048, 2048, 4096, 32, 8, 128
N_CORES = 8
HL = H // N_CORES      # 4 q-heads per core
EL = HL * D            # 512 local embed columns
f32 = mybir.dt.float32
f32r = mybir.dt.float32r

Sqrt = mybir.ActivationFunctionType.Sqrt
Ln = mybir.ActivationFunctionType.Ln
Exp = mybir.ActivationFunctionType.Exp


def _kernel_body(tc):
    nc = tc.nc
    # hid: [n-chunk][group of 4 k-tiles][partition][k-in-group][t] — 4
    # contraction tiles per DMA to amortize descriptor-issue cost.
    hid = nc.dram_tensor("hid", [4, 8, 128, 4, 512], f32r, kind="ExternalInput").ap()
    wq = nc.dram_tensor("wq", [8, 128, 16, 128], f32r, kind="ExternalInput").ap()
    kpp = nc.dram_tensor("kpp", [128, 2048], f32r, kind="ExternalInput").ap()
    vt = nc.dram_tensor("vt", [128, 16, 128], f32r, kind="ExternalInput").ap()
    wo = nc.dram_tensor("wo", [4, 128, 32, 128], f32r, kind="ExternalInput").ap()
    onesd = nc.dram_tensor("ones", [128, 128], f32r, kind="ExternalInput").ap()
    # outp: [m-group of 4][n][partition][m-in-group][t]
    outp = nc.dram_tensor("outp", [8, 4, 128, 4, 512], f32, kind="ExternalOutput").ap()

    with tc.tile_pool(name="persist", bufs=1) as persist, \
         tc.tile_pool(name="qtp", bufs=1) as qtp:
        ones = persist.tile([128, 128], f32r)
        nc.gpsimd.dma_start(out=ones, in_=onesd)
        eps_col = persist.tile([128, 1], f32)
        nc.vector.memset(eps_col, EPS)
        qT = qtp.tile([128, HL, 2048], f32r)      # q.T, heads side by side

        # ---- Phase 1: q-proj + fused per-chunk RMSNorm scale (in place) ----
        with tc.tile_pool(name="wqp", bufs=1) as wqp, \
             tc.tile_pool(name="hidp", bufs=12) as hidp, \
             tc.tile_pool(name="sqp", bufs=3) as sqp, \
             tc.tile_pool(name="srp", bufs=2) as srp, \
             tc.tile_pool(name="qps", bufs=4, space="PSUM") as qps, \
             tc.tile_pool(name="sps", bufs=2, space="PSUM") as sps:
            # wq halves: [m, half] -> lhsT tiles for 16 contraction k-tiles
            wq_sb = wqp.tile([128, 8, 16, 128], f32r)
            for mh in range(8):
                nc.gpsimd.dma_start(out=wq_sb[:, mh], in_=wq[mh])
            for n in range(4):
                cs = slice(n * 512, (n + 1) * 512)
                hts = []
                for g in range(8):
                    htg = hidp.tile([128, 4, 512], f32r, tag="hid")
                    nc.sync.dma_start(out=htg, in_=hid[n, g])
                    for j in range(4):
                        hts.append(htg[:, j, :])
                for m in range(4):
                    pq = qps.tile([128, 512], f32, tag="pq")
                    for k in range(32):
                        nc.tensor.matmul(pq, wq_sb[:, 2 * m + k // 16, k % 16, :],
                                         hts[k], start=(k == 0), stop=(k == 31))
                    # RMSNorm scale fused with the PSUM drain:
                    # qT = pq * rsqrt(mean_d(pq^2) + eps), rsqrt = exp(-ln/2)
                    sq = sqp.tile([128, 512], f32r)
                    nc.scalar.square(sq, pq)
                    ps = sps.tile([128, 512], f32, tag="sc")
                    nc.tensor.matmul(ps, ones, sq, start=True, stop=True)
                    lns = srp.tile([128, 512], f32)
                    nc.scalar.activation(lns, ps, Ln, scale=1.0 / D,
                                         bias=eps_col[:])
                    rinv = srp.tile([128, 512], f32)
                    nc.scalar.activation(rinv, lns, Exp, scale=-0.5)
                    nc.vector.tensor_mul(qT[:, m, cs], pq, rinv)

        # ---- Phases 2+3 ----
        with tc.tile_pool(name="wop", bufs=1) as wop, \
             tc.tile_pool(name="aop", bufs=1) as aop, \
             tc.tile_pool(name="rdp", bufs=2) as rdp, \
             tc.tile_pool(name="sump", bufs=3) as sump, \
             tc.tile_pool(name="expp", bufs=4) as expp:
            k_sb = wop.tile([128, 2048], f32r)
            nc.gpsimd.dma_start(out=k_sb, in_=kpp)
            v_sb = wop.tile([128, 16, 128], f32r)
            nc.gpsimd.dma_start(out=v_sb, in_=vt)
            wo_sb = wop.tile([128, 4, 32, 128], f32r)
            for k in range(4):
                nc.gpsimd.dma_start(out=wo_sb[:, k], in_=wo[k])
            aoT = aop.tile([128, HL, 2048], f32r)  # attn_out.T, local heads

            # Phase 2: attention, fully transposed
            with tc.tile_pool(name="scps", bufs=4, space="PSUM") as scps, \
                 tc.tile_pool(name="ops", bufs=2, space="PSUM") as ops, \
                 tc.tile_pool(name="dps", bufs=2, space="PSUM") as dps:
                for h in range(HL):
                    for tc_n in range(4):
                        cs = slice(tc_n * 512, (tc_n + 1) * 512)
                        po = ops.tile([128, 512], f32, tag="po")
                        pd = dps.tile([128, 512], f32, tag="pd")
                        for kp in range(8):
                            exs = []
                            for j in range(2):
                                kk = 2 * kp + j
                                pscr = scps.tile([128, 512], f32, tag="sc")
                                nc.tensor.matmul(pscr,
                                                 k_sb[:, kk * 128:(kk + 1) * 128],
                                                 qT[:, h, cs],
                                                 start=True, stop=True)
                                ex = expp.tile([128, 512], f32r)
                                nc.scalar.activation(ex, pscr, Exp)
                                nc.tensor.matmul(po, v_sb[:, kk, :], ex,
                                                 start=(kk == 0), stop=(kk == 15))
                                exs.append(ex)
                            # pre-sum the exp pair on DVE so the PE only runs
                            # 8 denominator matmuls per t-chunk instead of 16
                            exsum = sump.tile([128, 512], f32r)
                            nc.vector.tensor_add(exsum, exs[0], exs[1])
                            nc.tensor.matmul(pd, ones, exsum,
                                             start=(kp == 0), stop=(kp == 7))
                        rd = rdp.tile([128, 512], f32)
                        nc.vector.reciprocal(rd, pd)
                        nc.vector.tensor_mul(aoT[:, h, cs], po, rd)

            # Phase 3: o-projection partial  outT[j, t] = Wo_shard.T @ aoT
            with tc.tile_pool(name="obp", bufs=3) as obp, \
                 tc.tile_pool(name="pop", bufs=4, space="PSUM") as pop:
                for n in range(4):
                    cs = slice(n * 512, (n + 1) * 512)
                    for mg in range(8):
                        obg = obp.tile([128, 4, 512], f32, tag="ob")
                        for mj in range(4):
                            m = mg * 4 + mj
                            pp = pop.tile([128, 512], f32, tag="pq")
                            for k in range(4):
                                nc.tensor.matmul(pp, wo_sb[:, k, m, :],
                                                 aoT[:, k, cs],
                                                 start=(k == 0), stop=(k == 3))
                            nc.vector.tensor_copy(obg[:, mj, :], pp)
                        nc.sync.dma_start(out=outp[mg, n], in_=obg)


_NC_CACHE = None


def _build():
    global _NC_CACHE
    if _NC_CACHE is None:
        nc = bacc.Bacc("TRN2", target_bir_lowering=False, debug=False,
                       num_devices=N_CORES)
        with tile.TileContext(nc) as tc:
            _kernel_body(tc)
        nc.compile()
        _NC_CACHE = nc
    return _NC_CACHE


def _prepare_in_maps(hidden_states, k, v, Wq, Wo, q_norm_w, k_norm_w):
    hs = np.asarray(hidden_states, np.float32)
    k_ = np.asarray(k, np.float32)[0]      # [K, KVH, D]
    v_ = np.asarray(v, np.float32)[0]
    Wq_ = np.asarray(Wq, np.float32)
    Wo_ = np.asarray(Wo, np.float32)
    wqn = np.asarray(q_norm_w, np.float64)
    wkn = np.asarray(k_norm_w, np.float64)

    # Fold k-RMSNorm, both norm weights, and the attention scale into k''.
    kd = k_.astype(np.float64)
    rk = 1.0 / np.sqrt((kd ** 2).mean(-1, keepdims=True) + EPS)
    kpp_full = (kd * rk * (wqn * wkn) * (D ** -0.5)).astype(np.float32)

    hidT = np.ascontiguousarray(hs.T)                                  # [E, T]
    # [n, g, p, j, c] with k-tile index = 4g + j
    hid_tiles = np.ascontiguousarray(
        hidT.reshape(8, 4, 128, 4, 512).transpose(3, 0, 2, 1, 4))
    ones_arr = np.ones((128, 128), np.float32)

    in_maps = []
    for c in range(N_CORES):
        wqT = np.ascontiguousarray(Wq_[c * EL:(c + 1) * EL, :].T)      # [E, EL]
        # [mh, p, k16, c] with m = mh//2, contraction tile = (mh%2)*16 + k16
        wq_tiles = np.ascontiguousarray(
            wqT.reshape(2, 16, 128, 4, 128).transpose(3, 0, 2, 1, 4)
            .reshape(8, 128, 16, 128))
        woT = np.ascontiguousarray(Wo_[:, c * EL:(c + 1) * EL].T)      # [EL, E]
        wo_tiles = np.ascontiguousarray(
            woT.reshape(4, 128, 32, 128))                              # [k,p,m,c]
        kppT = np.ascontiguousarray(kpp_full[:, c, :].T)               # [D, K]
        v_tiles = np.ascontiguousarray(
            v_[:, c, :].reshape(16, 128, 128).transpose(1, 0, 2))      # [p,kk,d]
        in_maps.append({
            "hid": hid_tiles, "wq": wq_tiles, "kpp": kppT,
            "vt": v_tiles, "wo": wo_tiles, "ones": ones_arr,
        })
    return in_maps


def _gather(results):
    total = results[0]["outp"].astype(np.float32).copy()
    for r in results[1:]:
        total += r["outp"]
    # outp[mg, n, p, mj, c] -> outT[(mg*4+mj)*128+p, n*512+c]
    outT = total.transpose(0, 3, 2, 1, 4).reshape(E, T)
    return np.ascontiguousarray(outT.T)


def kernel(hidden_states, k, v, Wq, Wo, q_norm_w, k_norm_w):
    nc = _build()
    in_maps = _prepare_in_maps(hidden_states, k, v, Wq, Wo, q_norm_w, k_norm_w)
    res = bass_utils.run_bass_kernel_spmd(nc, in_maps,
                                          core_ids=list(range(N_CORES)))
    return _gather(res.results)


# revision 18
# speedup vs baseline: 1.0604x; 1.0604x over previous
"""TRN2 Bass kernel for nn_CombCrossAttention (GQA cross-attention block).

Computation (T=2048, K=2048, E=4096, H=32 q-heads, KVH=8 kv-heads, D=128):
    q  = hidden @ Wq.T;  per-head RMSNorm(q) * q_norm_w
    kn = RMSNorm(k) * k_norm_w  (GQA: each kv head serves 4 q heads)
    attn = softmax(qn @ kn.T / sqrt(D)) @ v
    out  = attn @ Wo.T

Sharding: tensor-parallel over heads on 8 NeuronCores. Core c owns q-heads
4c..4c+3 (Wq rows 512c..512c+512) and kv-head c, plus Wo columns
512c..512c+512; each core emits a [T, E] partial of the o-projection and
the host sums the 8 partials (the "all-reduce").

Device-side layout trick: everything is computed transposed ([feature, t])
so no on-chip transposes are needed anywhere:
  - q-proj emits qT [e', t] directly (lhsT = Wq shard already transposed)
  - scores are computed as scoresT [kk, t]; softmax's reductions over kk
    become ones-vector matmuls on the PE (cross-partition sums)
  - k-side RMSNorm, both norm weights and 1/sqrt(D) are folded into a
    host-precomputed k''; q-side RMSNorm becomes a per-t scale computed
    on-device from a ones-matmul + sqrt + reciprocal, applied in place
  - exp() needs no max-subtraction: post-RMSNorm scores are O(1)-bounded
Matmuls run in float32r (full PE rate, ~1.5e-4 max rel err).
"""
import sys

sys.path.insert(0, "/opt/trn_rl_repo")

import numpy as np

import jax
try:
    jax.config.update("jax_compilation_cache_dir", "/tmp/jax_neff_cache")
    jax.config.update("jax_persistent_cache_min_compile_time_secs", 1.0)
except Exception:
    pass

import concourse.bass as bass  # noqa: F401
import concourse.mybir as mybir
import concourse.tile as tile
from concourse import bacc, bass_utils

EPS = 1e-5
T, K, E, H, KVH, D = 2048, 2048, 4096, 32, 8, 128
N_CORES = 8
HL = H // N_CORES      # 4 q-heads per core
EL = HL * D            # 512 local embed columns
f32 = mybir.dt.float32
f32r = mybir.dt.float32r

Sqrt = mybir.ActivationFunctionType.Sqrt
Ln = mybir.ActivationFunctionType.Ln
Exp = mybir.ActivationFunctionType.Exp


def _kernel_body(tc):
    nc = tc.nc
    # hid: [n-chunk][group of 4 k-tiles][partition][k-in-group][t] — 4
    # contraction tiles per DMA to amortize descriptor-issue cost.
    hid = nc.dram_tensor("hid", [4, 8, 128, 4, 512], f32r, kind="ExternalInput").ap()
    wq = nc.dram_tensor("wq", [8, 128, 16, 128], f32r, kind="ExternalInput").ap()
    kpp = nc.dram_tensor("kpp", [128, 2048], f32r, kind="ExternalInput").ap()
    vt = nc.dram_tensor("vt", [128, 16, 128], f32r, kind="ExternalInput").ap()
    wo = nc.dram_tensor("wo", [4, 128, 32, 128], f32r, kind="ExternalInput").ap()
    onesd = nc.dram_tensor("ones", [128, 128], f32r, kind="ExternalInput").ap()
    # outp: [m-group of 4][n][partition][m-in-group][t]
    outp = nc.dram_tensor("outp", [8, 4, 128, 4, 512], f32, kind="ExternalOutput").ap()

    with tc.tile_pool(name="persist", bufs=1) as persist, \
         tc.tile_pool(name="qtp", bufs=1) as qtp:
        ones = persist.tile([128, 128], f32r)
        nc.gpsimd.dma_start(out=ones, in_=onesd)
        eps_col = persist.tile([128, 1], f32)
        nc.vector.memset(eps_col, EPS)
        qT = qtp.tile([128, HL, 2048], f32r)      # q.T, heads side by side

        # ---- Phase 1: q-proj + fused per-chunk RMSNorm scale (in place) ----
        with tc.tile_pool(name="wqp", bufs=1) as wqp, \
             tc.tile_pool(name="hidp", bufs=12) as hidp, \
             tc.tile_pool(name="sqp", bufs=3) as sqp, \
             tc.tile_pool(name="srp", bufs=2) as srp, \
             tc.tile_pool(name="qps", bufs=4, space="PSUM") as qps, \
             tc.tile_pool(name="sps", bufs=2, space="PSUM") as sps:
            # wq halves: [m, half] -> lhsT tiles for 16 contraction k-tiles
            wq_sb = wqp.tile([128, 8, 16, 128], f32r)
            for mh in range(8):
                nc.gpsimd.dma_start(out=wq_sb[:, mh], in_=wq[mh])
            for n in range(4):
                cs = slice(n * 512, (n + 1) * 512)
                hts = []
                for g in range(8):
                    htg = hidp.tile([128, 4, 512], f32r, tag="hid")
                    nc.sync.dma_start(out=htg, in_=hid[n, g])
                    for j in range(4):
                        hts.append(htg[:, j, :])
                for m in range(4):
                    pq = qps.tile([128, 512], f32, tag="pq")
                    for k in range(32):
                        nc.tensor.matmul(pq, wq_sb[:, 2 * m + k // 16, k % 16, :],
                                         hts[k], start=(k == 0), stop=(k == 31))
                    # RMSNorm scale fused with the PSUM drain:
                    # qT = pq * rsqrt(mean_d(pq^2) + eps), rsqrt = exp(-ln/2)
                    sq = sqp.tile([128, 512], f32r)
                    nc.scalar.square(sq, pq)
                    ps = sps.tile([128, 512], f32, tag="sc")
                    nc.tensor.matmul(ps, ones, sq, start=True, stop=True)
                    lns = srp.tile([128, 512], f32)
                    nc.scalar.activation(lns, ps, Ln, scale=1.0 / D,
                                         bias=eps_col[:])
                    rinv = srp.tile([128, 512], f32)
                    nc.scalar.activation(rinv, lns, Exp, scale=-0.5)
                    nc.vector.tensor_mul(qT[:, m, cs], pq, rinv)

        # ---- Phases 2+3 ----
        with tc.tile_pool(name="wop", bufs=1) as wop, \
             tc.tile_pool(name="aop", bufs=1) as aop, \
             tc.tile_pool(name="rdp", bufs=2) as rdp, \
             tc.tile_pool(name="expp", bufs=4) as expp:
            k_sb = wop.tile([128, 2048], f32r)
            nc.gpsimd.dma_start(out=k_sb, in_=kpp)
            v_sb = wop.tile([128, 16, 128], f32r)
            nc.gpsimd.dma_start(out=v_sb, in_=vt)
            wo_sb = wop.tile([128, 4, 32, 128], f32r)
            for k in range(4):
                nc.gpsimd.dma_start(out=wo_sb[:, k], in_=wo[k])
            aoT = aop.tile([128, HL, 2048], f32r)  # attn_out.T, local heads

            # Phase 2: attention, fully transposed
            with tc.tile_pool(name="scps", bufs=4, space="PSUM") as scps, \
                 tc.tile_pool(name="ops", bufs=2, space="PSUM") as ops, \
                 tc.tile_pool(name="dps", bufs=2, space="PSUM") as dps:
                for h in range(HL):
                    for tc_n in range(4):
                        cs = slice(tc_n * 512, (tc_n + 1) * 512)
                        po = ops.tile([128, 512], f32, tag="po")
                        pd = dps.tile([128, 512], f32, tag="pd")
                        for kk in range(16):
                            pscr = scps.tile([128, 512], f32, tag="sc")
                            nc.tensor.matmul(pscr, k_sb[:, kk * 128:(kk + 1) * 128],
                                             qT[:, h, cs], start=True, stop=True)
                            ex = expp.tile([128, 512], f32r)
                            nc.scalar.activation(ex, pscr, Exp)
                            nc.tensor.matmul(po, v_sb[:, kk, :], ex,
                                             start=(kk == 0), stop=(kk == 15))
                            nc.tensor.matmul(pd, ones, ex,
                                             start=(kk == 0), stop=(kk == 15))
                        rd = rdp.tile([128, 512], f32)
                        nc.vector.reciprocal(rd, pd)
                        nc.vector.tensor_mul(aoT[:, h, cs], po, rd)

            # Phase 3: o-projection partial  outT[j, t] = Wo_shard.T @ aoT
            with tc.tile_pool(name="obp", bufs=3) as obp, \
                 tc.tile_pool(name="pop", bufs=4, space="PSUM") as pop:
                for n in range(4):
                    cs = slice(n * 512, (n + 1) * 512)
                    for mg in range(8):
                        obg = obp.tile([128, 4, 512], f32, tag="ob")
                        for mj in range(4):
                            m = mg * 4 + mj
                            pp = pop.tile([128, 512], f32, tag="pq")
                            for k in range(4):
                                nc.tensor.matmul(pp, wo_sb[:, k, m, :],
                                                 aoT[:, k, cs],
                                                 start=(k == 0), stop=(k == 3))
                            nc.vector.tensor_copy(obg[:, mj, :], pp)
                        nc.sync.dma_start(out=outp[mg, n], in_=obg)


_NC_CACHE = None


def _build():
    global _NC_CACHE
    if _NC_CACHE is None:
        nc = bacc.Bacc("TRN2", target_bir_lowering=False, debug=False,
                       num_devices=N_CORES)
        with tile.TileContext(nc) as tc:
            _kernel_body(tc)
        nc.compile()
        _NC_CACHE = nc
    return _NC_CACHE


def _prepare_in_maps(hidden_states, k, v, Wq, Wo, q_norm_w, k_norm_w):
    hs = np.asarray(hidden_states, np.float32)
    k_ = np.asarray(k, np.float32)[0]      # [K, KVH, D]
    v_ = np.asarray(v, np.float32)[0]
    Wq_ = np.asarray(Wq, np.float32)
    Wo_ = np.asarray(Wo, np.float32)
    wqn = np.asarray(q_norm_w, np.float64)
    wkn = np.asarray(k_norm_w, np.float64)

    # Fold k-RMSNorm, both norm weights, and the attention scale into k''.
    kd = k_.astype(np.float64)
    rk = 1.0 / np.sqrt((kd ** 2).mean(-1, keepdims=True) + EPS)
    kpp_full = (kd * rk * (wqn * wkn) * (D ** -0.5)).astype(np.float32)

    hidT = np.ascontiguousarray(hs.T)                                  # [E, T]
    # [n, g, p, j, c] with k-tile index = 4g + j
    hid_tiles = np.ascontiguousarray(
        hidT.reshape(8, 4, 128, 4, 512).transpose(3, 0, 2, 1, 4))
    ones_arr = np.ones((128, 128), np.float32)

    in_maps = []
    for c in range(N_CORES):
        wqT = np.ascontiguousarray(Wq_[c * EL:(c + 1) * EL, :].T)      # [E, EL]
        # [mh, p, k16, c] with m = mh//2, contraction tile = (mh%2)*16 + k16
        wq_tiles = np.ascontiguousarray(
            wqT.reshape(2, 16, 128, 4, 128).transpose(3, 0, 2, 1, 4)
            .reshape(8, 128, 16, 128))
        woT = np.ascontiguousarray(Wo_[:, c * EL:(c + 1) * EL].T)      # [EL, E]
        wo_tiles = np.ascontiguousarray(
            woT.reshape(4, 128, 32, 128))                              # [k,p,m,c]
        kppT = np.ascontiguousarray(kpp_full[:, c, :].T)               # [D, K]
        v_tiles = np.ascontiguousarray(
            v_[:, c, :].reshape(16, 128, 128).transpose(1, 0, 2))      # [p,kk,d]
        in_maps.append({
            "hid": hid_tiles, "wq": wq_tiles, "kpp": kppT,
            "vt": v_tiles, "wo": wo_tiles, "ones": ones_arr,
        })
    return in_maps


def _gather(results):
    total = results[0]["outp"].astype(np.float32).copy()
    for r in results[1:]:
        total += r["outp"]
    # outp[mg, n, p, mj, c] -> outT[(mg*4+mj)*128+p, n*512+c]
    outT = total.transpose(0, 3, 2, 1, 4).reshape(E, T)
    return np.ascontiguousarray(outT.T)


def kernel(hidden_states, k, v, Wq, Wo, q_norm_w, k_norm_w):
    nc = _build()
    in_maps = _prepare_in_maps(hidden_states, k, v, Wq, Wo, q_norm_w, k_norm_w)
    res = bass_utils.run_bass_kernel_spmd(nc, in_maps,
                                          core_ids=list(range(N_CORES)))
    return _gather(res.results)
